# revision 11
# baseline (speedup 1.0000x reference)
"""Trainium2 Bass kernel for quantized multi-head attention (ViT-shape).

Computation (per reference):
  q/k/v = x @ W{q,k,v}.T ; per-head scores = (q k^T) * D^-0.5 ;
  fake_quant_per_head(scores) ; softmax ; out = attn @ v ;
  fake_quant_per_head(out) ; merge heads ; out @ Wo.T + bo.

Sharding: data-parallel over batch, 8 images per core on 8 NeuronCores.

Key device-side design (per core, 8 images = 1576 tokens):
  - All weights host-transposed to [d_in, d_out] layout; quant scale factors
    folded into Wq (alpha/s_attn per head) and Wo (s_out per head); quant
    zero-offset (lo) folded into the output bias host-side.
  - q,k computed feature-major qT/kT [768, t] (heads on partitions) so the
    scores matmul contracts over d_head directly.
  - Scores computed transposed: ST[j, i] (j = key token on partitions). The
    fake-quant is ONE 2-op tensor_scalar: clip(min,max) + int16-convert
    (convert truncates toward zero == torch trunc). exp via ACT from int16
    with scale=s (the +lo offset cancels in softmax).
  - softmax denominator comes free from the P@V matmul: v is stored with an
    extra per-head column holding s_out[h]; column 64 of the PV output is
    s_out*sum_j(E) which is exactly the reciprocal argument needed for the
    normalized+pre-divided out-quant.
  - out-quant: TS(mult 1/denom', min hi/s) -> TS(max lo/s -> int16) ->
    convert; integer-valued Oq feeds the output projection; PE transpose
    (with identity) converts Oq to feature-major for the Wo matmul.
  - fp32 everywhere by default; KVAR env selects faster dtype variants.
"""

import os
import numpy as np

B, N, D, H = 64, 197, 768, 12
DH = D // H  # 64
NCORES = 8
BPC = B // NCORES          # 8 images per core
T = BPC * N                # 1576 tokens per core
IMGS_PER_CHUNK = 2
NCHUNK = BPC // IMGS_PER_CHUNK  # 4
TC = IMGS_PER_CHUNK * N    # 394 tokens per chunk
KT = D // 128              # 6 d-tiles
OT = D // 128              # 6 o-tiles
Q_LEVELS = 255

_RUNNER_CACHE = {}


def _head_off(h):
    # per-image wide PV psum [128, 1024] (2 banks): heads 0-6 in bank 0,
    # heads 7-11 in bank 1 (a 65-wide block may not cross a 512-f32 bank).
    return 65 * h if h < 7 else 512 + 65 * (h - 7)


def _build_program(hi_s_attn, lo_s_attn, s_attn, hi_s_out, lo_s_out, s_out, variant):
    import concourse.bass as bass
    import concourse.bacc as bacc
    import concourse.mybir as mybir
    from concourse.tile import TileContext

    f32 = mybir.dt.float32
    f32r = mybir.dt.float32r
    bf16 = mybir.dt.bfloat16
    i16 = mybir.dt.int16

    # w_dt: dtype of DMA-loaded projection operands (wq/wk/wv, xT).
    # attn_dt: dtype of on-device-written matmul operands (q/k/E/v/Oq/OT)
    # and of wo (wo must match OT for the output projection).
    if variant == "f32":
        w_dt, attn_dt = f32, f32
    elif variant == "f32r":
        w_dt, attn_dt = f32r, f32
    elif variant == "bf16":
        w_dt, attn_dt = f32r, bf16
    else:
        raise ValueError(variant)
    wo_dt = attn_dt

    nc = bacc.Bacc("TRN2", target_bir_lowering=False, debug=False)

    xT_d = nc.dram_tensor("xT", [D, T], w_dt, kind="ExternalInput").ap()
    wq_d = nc.dram_tensor("wqts", [D, D], w_dt, kind="ExternalInput").ap()
    wk_d = nc.dram_tensor("wkt", [D, D], w_dt, kind="ExternalInput").ap()
    wv_d = nc.dram_tensor("wvt", [D, D], w_dt, kind="ExternalInput").ap()
    wo_d = nc.dram_tensor("wots", [D, D], wo_dt, kind="ExternalInput").ap()
    bo_d = nc.dram_tensor("bof", [D], f32, kind="ExternalInput").ap()
    id_d = nc.dram_tensor("ident", [128, 128], attn_dt, kind="ExternalInput").ap()
    out_d = nc.dram_tensor("outT", [D, T], f32, kind="ExternalOutput").ap()

    Exp = mybir.ActivationFunctionType.Exp
    Ident = mybir.ActivationFunctionType.Identity
    A = mybir.AluOpType

    with TileContext(nc) as tc:
        with (
            tc.tile_pool(name="const", bufs=1) as cpool,
            tc.tile_pool(name="sb", bufs=2) as sb,
            tc.tile_pool(name="ps", bufs=2, space="PSUM") as ps,
        ):
            # ---- resident constants ----
            wq_sb, wk_sb, wv_sb, wo_sb = [], [], [], []
            for k in range(KT):
                t_q = cpool.tile([128, D], w_dt, name=f"wq{k}")
                nc.sync.dma_start(out=t_q, in_=wq_d[128 * k:128 * (k + 1), :])
                wq_sb.append(t_q)
                t_k = cpool.tile([128, D], w_dt, name=f"wk{k}")
                nc.sync.dma_start(out=t_k, in_=wk_d[128 * k:128 * (k + 1), :])
                wk_sb.append(t_k)
                t_v = cpool.tile([128, D], w_dt, name=f"wv{k}")
                nc.sync.dma_start(out=t_v, in_=wv_d[128 * k:128 * (k + 1), :])
                wv_sb.append(t_v)
                t_o = cpool.tile([128, D], wo_dt, name=f"wo{k}")
                nc.sync.dma_start(out=t_o, in_=wo_d[128 * k:128 * (k + 1), :])
                wo_sb.append(t_o)
            bo_sb = cpool.tile([128, OT], f32, name="bo")
            for k in range(OT):
                nc.sync.dma_start(
                    out=bo_sb[:, k:k + 1],
                    in_=bo_d[128 * k:128 * (k + 1)].rearrange("(p o) -> p o", o=1),
                )
            ident = cpool.tile([128, 128], attn_dt, name="ident")
            nc.sync.dma_start(out=ident, in_=id_d)

            for c in range(NCHUNK):
                c0 = TC * c
                # ---- load xT chunk ----
                xc = sb.tile([128, KT * TC], w_dt, name=f"xc{c}", tag="xc")
                for k in range(KT):
                    nc.sync.dma_start(
                        out=xc[:, TC * k:TC * (k + 1)],
                        in_=xT_d[128 * k:128 * (k + 1), c0:c0 + TC],
                    )

                # ---- q/k projections (feature-major) ----
                qc = sb.tile([128, OT * TC], attn_dt, name=f"qc{c}", tag="qc")
                kc = sb.tile([128, OT * TC], attn_dt, name=f"kc{c}", tag="kc")
                for (wsb, dst) in ((wq_sb, qc), (wk_sb, kc)):
                    for o in range(OT):
                        pj = ps.tile([128, TC], f32, name=f"pj{c}{o}", tag="proj")
                        for k in range(KT):
                            nc.tensor.matmul(
                                pj,
                                lhsT=wsb[k][:, 128 * o:128 * (o + 1)],
                                rhs=xc[:, TC * k:TC * (k + 1)],
                                start=(k == 0), stop=(k == KT - 1),
                            )
                        nc.scalar.activation(dst[:, TC * o:TC * (o + 1)], pj, Ident)

                # ---- v projection (token-major, per-head 65-col blocks) ----
                vaug = []
                for im in range(IMGS_PER_CHUNK):
                    for tt in range(2):
                        tl = 128 if tt == 0 else N - 128
                        va = sb.tile([128, H * 65], attn_dt,
                                     name=f"va{c}{im}{tt}", tag="vaug", bufs=4)
                        vav = va.rearrange("p (h c) -> p h c", c=65)
                        for oc in range(2):
                            vp = ps.tile([128, 384], f32,
                                         name=f"vp{c}{im}{tt}{oc}", tag="proj")
                            for k in range(KT):
                                nc.tensor.matmul(
                                    vp[:tl],
                                    lhsT=xc[:, TC * k + N * im + 128 * tt:
                                            TC * k + N * im + 128 * tt + tl],
                                    rhs=wv_sb[k][:, 384 * oc:384 * (oc + 1)],
                                    start=(k == 0), stop=(k == KT - 1),
                                )
                            nc.vector.tensor_copy(
                                vav[:tl, 6 * oc:6 * (oc + 1), 0:64],
                                vp[:tl].rearrange("p (h c) -> p h c", c=64),
                            )
                        for h in range(H):
                            nc.gpsimd.memset(vav[:tl, h, 64:65], float(s_out[h]))
                        vaug.append(va)

                # ---- attention per image ----
                oqf_all = []
                for im in range(IMGS_PER_CHUNK):
                    pv = []
                    for it in range(2):
                        pvt = ps.tile([128, 1024], f32,
                                      name=f"pv{c}{im}{it}", tag="pv")
                        pv.append(pvt)
                    for h in range(H):
                        o, row = h // 2, (h % 2) * 64
                        base = TC * o + N * im
                        efs = []
                        for jt in range(2):
                            jl = 128 if jt == 0 else N - 128
                            sp = ps.tile([128, N], f32,
                                         name=f"sp{c}{im}{h}{jt}", tag="st")
                            nc.tensor.matmul(
                                sp[:jl],
                                lhsT=kc[row:row + 64,
                                        base + 128 * jt:base + 128 * jt + jl],
                                rhs=qc[row:row + 64, base:base + N],
                                start=True, stop=True,
                            )
                            q16 = sb.tile([128, N], i16,
                                          name=f"q16{c}{im}{h}{jt}", tag="q16", bufs=4)
                            nc.vector.tensor_scalar(
                                out=q16[:jl], in0=sp[:jl],
                                scalar1=float(hi_s_attn[h]),
                                scalar2=float(lo_s_attn[h]),
                                op0=A.min, op1=A.max,
                            )
                            ef = sb.tile([128, N], attn_dt,
                                         name=f"ef{c}{im}{h}{jt}", tag="ef", bufs=4)
                            nc.scalar.activation(ef[:jl], q16[:jl], Exp,
                                                 scale=float(s_attn[h]))
                            efs.append(ef)
                        for it in range(2):
                            il = 128 if it == 0 else N - 128
                            off = _head_off(h)
                            for jt in range(2):
                                jl = 128 if jt == 0 else N - 128
                                nc.tensor.matmul(
                                    pv[it][:il, off:off + 65],
                                    lhsT=efs[jt][:jl, 128 * it:128 * it + il],
                                    rhs=vaug[2 * im + jt].rearrange(
                                        "p (h c) -> p h c", c=65)[:jl, h, :],
                                    start=(jt == 0), stop=(jt == 1),
                                )
                    # normalization + out-quant
                    for it in range(2):
                        il = 128 if it == 0 else N - 128
                        inv = sb.tile([128, H], f32, name=f"inv{c}{im}{it}",
                                      tag="inv", bufs=4)
                        bank0 = pv[it][:, 0:65 * 7].rearrange(
                            "p (h c) -> p h c", c=65)
                        bank1 = pv[it][:, 512:512 + 65 * 5].rearrange(
                            "p (h c) -> p h c", c=65)
                        nc.vector.reciprocal(inv[:il, 0:7], bank0[:il, :, 64])
                        nc.vector.reciprocal(inv[:il, 7:12], bank1[:il, :, 64])
                        tmp = sb.tile([128, D], f32, name=f"oqt{c}{im}{it}",
                                      tag="oqt")
                        oqi = sb.tile([128, D], i16, name=f"oqi{c}{im}{it}",
                                      tag="oqi")
                        for h in range(H):
                            off = _head_off(h)
                            nc.vector.tensor_scalar(
                                out=tmp[:il, 64 * h:64 * (h + 1)],
                                in0=pv[it][:il, off:off + 64],
                                scalar1=inv[:il, h:h + 1],
                                scalar2=float(hi_s_out[h]),
                                op0=A.mult, op1=A.min,
                            )
                            nc.vector.tensor_scalar(
                                out=oqi[:il, 64 * h:64 * (h + 1)],
                                in0=tmp[:il, 64 * h:64 * (h + 1)],
                                scalar1=float(lo_s_out[h]), scalar2=None,
                                op0=A.max,
                            )
                        oqf = sb.tile([128, D], attn_dt, name=f"oqf{c}{im}{it}",
                                      tag="oqf", bufs=4)
                        nc.vector.tensor_copy(oqf[:il], oqi[:il])
                        oqf_all.append(oqf)

                # ---- transpose Oq -> feature-major OT chunk ----
                otc = sb.tile([128, KT * TC], attn_dt, name=f"otc{c}", tag="otc",
                              bufs=1)
                for k in range(KT):
                    tp = ps.tile([128, TC], f32, name=f"tp{c}{k}", tag="proj")
                    for im in range(IMGS_PER_CHUNK):
                        for it in range(2):
                            il = 128 if it == 0 else N - 128
                            coff = N * im + 128 * it
                            nc.tensor.transpose(
                                tp[:, coff:coff + il],
                                oqf_all[2 * im + it][:il, 128 * k:128 * (k + 1)],
                                ident[:il, :il],
                            )
                    nc.scalar.activation(otc[:, TC * k:TC * (k + 1)], tp, Ident)

                # ---- output projection ----
                for o in range(OT):
                    op_ = ps.tile([128, TC], f32, name=f"op{c}{o}", tag="proj")
                    for k in range(KT):
                        nc.tensor.matmul(
                            op_,
                            lhsT=wo_sb[k][:, 128 * o:128 * (o + 1)],
                            rhs=otc[:, TC * k:TC * (k + 1)],
                            start=(k == 0), stop=(k == KT - 1),
                        )
                    osb = sb.tile([128, TC], f32, name=f"osb{c}{o}", tag="osb",
                                  bufs=3)
                    nc.scalar.activation(osb, op_, Ident, bias=bo_sb[:, o:o + 1])
                    nc.sync.dma_start(
                        out=out_d[128 * o:128 * (o + 1), c0:c0 + TC], in_=osb
                    )
    nc.compile()
    return nc


def _prepare_host_inputs(x, Wq, Wk, Wv, Wo, bo,
                         qmin_attn, qmax_attn, qmin_out, qmax_out, variant):
    """Returns (in_maps list per core, qparam tuple)."""
    f = np.float32
    alpha = np.float32(D ** -0.5)
    s_attn = ((qmax_attn - qmin_attn) / Q_LEVELS).astype(f)
    s_out = ((qmax_out - qmin_out) / Q_LEVELS).astype(f)
    hi_s_attn = (qmax_attn / s_attn).astype(f)
    lo_s_attn = (qmin_attn / s_attn).astype(f)
    hi_s_out = (qmax_out / s_out).astype(f)
    lo_s_out = (qmin_out / s_out).astype(f)

    head_of_o = np.arange(D) // DH
    wqts = np.ascontiguousarray(
        (Wq * (alpha / s_attn[head_of_o])[:, None]).T).astype(f)
    wkt = np.ascontiguousarray(Wk.T).astype(f)
    wvt = np.ascontiguousarray(Wv.T).astype(f)
    wots = np.ascontiguousarray((Wo * s_out[head_of_o][None, :]).T).astype(f)
    bof = (bo + Wo @ qmin_out[head_of_o]).astype(f)

    if variant == "bf16":
        import ml_dtypes
        adt = ml_dtypes.bfloat16
        wots_c, ident = wots.astype(adt), np.eye(128, dtype=adt)
    else:
        wots_c, ident = wots, np.eye(128, dtype=f)

    in_maps = []
    for i in range(NCORES):
        xs = np.ascontiguousarray(
            x[BPC * i:BPC * (i + 1)].reshape(T, D).T).astype(f)
        in_maps.append(dict(xT=xs, wqts=wqts, wkt=wkt, wvt=wvt, wots=wots_c,
                            bof=bof, ident=ident))
    qparams = (hi_s_attn, lo_s_attn, s_attn, hi_s_out, lo_s_out, s_out)
    return in_maps, qparams


class _Runner:
    """Compiled SPMD executable over 8 cores (PJRT path, jit cached)."""

    def __init__(self, nc):
        import jax
        import concourse.mybir as mybir
        from concourse import bass2jax
        from jax.sharding import Mesh, PartitionSpec
        from jax.experimental.shard_map import shard_map

        bass2jax.install_neuronx_cc_hook()
        self.nc = nc
        assert nc.dbg_addr is None
        partition_name = (nc.partition_id_tensor.name
                          if nc.partition_id_tensor else None)

        in_names, out_names, out_avals, zero_outs = [], [], [], []
        for alloc in nc.m.functions[0].allocations:
            if not isinstance(alloc, mybir.MemoryLocationSet):
                continue
            name = alloc.memorylocations[0].name
            if alloc.kind == "ExternalInput":
                if name != partition_name:
                    in_names.append(name)
            elif alloc.kind == "ExternalOutput":
                shape = tuple(alloc.tensor_shape)
                dtype = mybir.dt.np(alloc.dtype)
                out_names.append(name)
                out_avals.append(jax.core.ShapedArray(shape, dtype))
                zero_outs.append(np.zeros(shape, dtype))
        self.in_names, self.out_names = in_names, out_names
        self.out_avals, self.zero_outs = out_avals, zero_outs
        n_params, n_outs = len(in_names), len(out_avals)
        all_names = list(in_names) + list(out_names)
        if partition_name is not None:
            all_names.append(partition_name)
        all_names = tuple(all_names)

        def _body(*args):
            operands = list(args)
            if partition_name is not None:
                operands.append(bass2jax.partition_id_tensor())
            outs = bass2jax._bass_exec_p.bind(
                *operands,
                out_avals=tuple(out_avals),
                in_names=all_names,
                out_names=tuple(out_names),
                lowering_input_output_aliases=(),
                sim_require_finite=True,
                sim_require_nnan=True,
                nc=nc,
            )
            return tuple(outs)

        devices = jax.devices()[:NCORES]
        mesh = Mesh(np.asarray(devices), ("core",))
        self.sharded = jax.jit(
            shard_map(_body, mesh=mesh,
                      in_specs=(PartitionSpec("core"),) * (n_params + n_outs),
                      out_specs=(PartitionSpec("core"),) * n_outs,
                      check_rep=False),
            donate_argnums=tuple(range(n_params, n_params + n_outs)),
            keep_unused=True,
        )

    def concat_inputs(self, in_maps):
        return [np.concatenate([np.asarray(m[name]) for m in in_maps], axis=0)
                for name in self.in_names]

    def run_raw(self, concat_in):
        concat_zeros = [np.zeros((NCORES * z.shape[0], *z.shape[1:]), z.dtype)
                        for z in self.zero_outs]
        return self.sharded(*concat_in, *concat_zeros)

    def run(self, in_maps):
        out_arrs = self.run_raw(self.concat_inputs(in_maps))
        return [
            {name: np.asarray(out_arrs[i]).reshape(
                NCORES, *self.out_avals[i].shape)[c]
             for i, name in enumerate(self.out_names)}
            for c in range(NCORES)
        ]


def get_runner(qparams, variant):
    key = (variant,) + tuple(p.tobytes() for p in qparams)
    if key not in _RUNNER_CACHE:
        _RUNNER_CACHE[key] = _Runner(_build_program(*qparams, variant))
    return _RUNNER_CACHE[key]


def kernel(x, Wq, Wk, Wv, Wo, bo, qmin_attn, qmax_attn, qmin_out, qmax_out):
    variant = os.environ.get("KVAR", "f32")
    in_maps, qparams = _prepare_host_inputs(
        np.asarray(x, np.float32), np.asarray(Wq, np.float32),
        np.asarray(Wk, np.float32), np.asarray(Wv, np.float32),
        np.asarray(Wo, np.float32), np.asarray(bo, np.float32),
        np.asarray(qmin_attn, np.float32), np.asarray(qmax_attn, np.float32),
        np.asarray(qmin_out, np.float32), np.asarray(qmax_out, np.float32),
        variant,
    )
    runner = get_runner(qparams, variant)
    results = runner.run(in_maps)
    out = np.empty((B, N, D), np.float32)
    for i in range(NCORES):
        out[BPC * i:BPC * (i + 1)] = results[i]["outT"].T.reshape(BPC, N, D)
    kernel.last_runner = runner
    kernel.last_in_maps = in_maps
    return out


# revision 13
# speedup vs baseline: 91.8464x; 91.8464x over previous
"""Trainium2 Bass kernel for quantized multi-head attention (ViT-shape).

Computation (per reference):
  q/k/v = x @ W{q,k,v}.T ; per-head scores = (q k^T) * D^-0.5 ;
  fake_quant_per_head(scores) ; softmax ; out = attn @ v ;
  fake_quant_per_head(out) ; merge heads ; out @ Wo.T + bo.

Sharding: data-parallel over batch, 8 images per core on 8 NeuronCores.

Key device-side design (per core, 8 images = 1576 tokens):
  - All weights host-transposed to [d_in, d_out] layout; quant scale factors
    folded into Wq (alpha/s_attn per head) and Wo (s_out per head); quant
    zero-offset (lo) folded into the output bias host-side.
  - q,k computed feature-major qT/kT [768, t] (heads on partitions) so the
    scores matmul contracts over d_head directly.
  - Scores computed transposed: ST[j, i] (j = key token on partitions). The
    fake-quant is ONE 2-op tensor_scalar: clip(min,max) + int16-convert
    (convert truncates toward zero == torch trunc). exp via ACT from int16
    with scale=s (the +lo offset cancels in softmax).
  - softmax denominator comes free from the P@V matmul: v is stored with an
    extra per-head column holding s_out[h]; column 64 of the PV output is
    s_out*sum_j(E) which is exactly the reciprocal argument needed for the
    normalized+pre-divided out-quant.
  - out-quant: TS(mult 1/denom', min hi/s) -> TS(max lo/s -> int16) ->
    convert; integer-valued Oq feeds the output projection; PE transpose
    (with identity) converts Oq to feature-major for the Wo matmul.
  - fp32 everywhere by default; KVAR env selects faster dtype variants.
"""

import os
import numpy as np

B, N, D, H = 64, 197, 768, 12
DH = D // H  # 64
NCORES = 8
BPC = B // NCORES          # 8 images per core
T = BPC * N                # 1576 tokens per core
IMGS_PER_CHUNK = 2
NCHUNK = BPC // IMGS_PER_CHUNK  # 4
TC = IMGS_PER_CHUNK * N    # 394 tokens per chunk
KT = D // 128              # 6 d-tiles
OT = D // 128              # 6 o-tiles
Q_LEVELS = 255

_RUNNER_CACHE = {}


def _head_off(h):
    # per-image wide PV psum [128, 1024] (2 banks): heads 0-6 in bank 0,
    # heads 7-11 in bank 1 (a 65-wide block may not cross a 512-f32 bank).
    return 65 * h if h < 7 else 512 + 65 * (h - 7)


def _build_program(hi_s_attn, lo_s_attn, s_attn, hi_s_out, lo_s_out, s_out, variant):
    import concourse.bass as bass
    import concourse.bacc as bacc
    import concourse.mybir as mybir
    from concourse.tile import TileContext

    f32 = mybir.dt.float32
    f32r = mybir.dt.float32r
    bf16 = mybir.dt.bfloat16
    i16 = mybir.dt.int16

    # w_dt: dtype of DMA-loaded projection operands (wq/wk/wv, xT).
    # attn_dt: dtype of on-device-written matmul operands (q/k/E/v/Oq/OT)
    # and of wo (wo must match OT for the output projection).
    if variant == "f32":
        w_dt, attn_dt = f32, f32
    elif variant == "f32r":
        w_dt, attn_dt = f32r, f32
    elif variant == "bf16":
        w_dt, attn_dt = f32r, bf16
    else:
        raise ValueError(variant)
    wo_dt = attn_dt

    nc = bacc.Bacc("TRN2", target_bir_lowering=False, debug=False)

    xT_d = nc.dram_tensor("xT", [D, T], w_dt, kind="ExternalInput").ap()
    wq_d = nc.dram_tensor("wqts", [D, D], w_dt, kind="ExternalInput").ap()
    wk_d = nc.dram_tensor("wkt", [D, D], w_dt, kind="ExternalInput").ap()
    wv_d = nc.dram_tensor("wvt", [D, D], w_dt, kind="ExternalInput").ap()
    wo_d = nc.dram_tensor("wots", [D, D], wo_dt, kind="ExternalInput").ap()
    bo_d = nc.dram_tensor("bof", [D], f32, kind="ExternalInput").ap()
    id_d = nc.dram_tensor("ident", [128, 128], attn_dt, kind="ExternalInput").ap()
    out_d = nc.dram_tensor("outT", [D, T], f32, kind="ExternalOutput").ap()

    Exp = mybir.ActivationFunctionType.Exp
    Ident = mybir.ActivationFunctionType.Identity
    A = mybir.AluOpType

    with TileContext(nc) as tc:
        with (
            tc.tile_pool(name="const", bufs=1) as cpool,
            tc.tile_pool(name="sb", bufs=2) as sb,
            tc.tile_pool(name="ps", bufs=2, space="PSUM") as ps,
        ):
            # ---- resident constants ----
            wq_sb, wk_sb, wv_sb, wo_sb = [], [], [], []
            for k in range(KT):
                t_q = cpool.tile([128, D], w_dt, name=f"wq{k}")
                nc.sync.dma_start(out=t_q, in_=wq_d[128 * k:128 * (k + 1), :])
                wq_sb.append(t_q)
                t_k = cpool.tile([128, D], w_dt, name=f"wk{k}")
                nc.sync.dma_start(out=t_k, in_=wk_d[128 * k:128 * (k + 1), :])
                wk_sb.append(t_k)
                t_v = cpool.tile([128, D], w_dt, name=f"wv{k}")
                nc.sync.dma_start(out=t_v, in_=wv_d[128 * k:128 * (k + 1), :])
                wv_sb.append(t_v)
                t_o = cpool.tile([128, D], wo_dt, name=f"wo{k}")
                nc.sync.dma_start(out=t_o, in_=wo_d[128 * k:128 * (k + 1), :])
                wo_sb.append(t_o)
            bo_sb = cpool.tile([128, OT], f32, name="bo")
            for k in range(OT):
                nc.sync.dma_start(
                    out=bo_sb[:, k:k + 1],
                    in_=bo_d[128 * k:128 * (k + 1)].rearrange("(p o) -> p o", o=1),
                )
            ident = cpool.tile([128, 128], attn_dt, name="ident")
            nc.sync.dma_start(out=ident, in_=id_d)

            for c in range(NCHUNK):
                c0 = TC * c
                # ---- load xT chunk ----
                xc = sb.tile([128, KT * TC], w_dt, name=f"xc{c}", tag="xc")
                for k in range(KT):
                    nc.sync.dma_start(
                        out=xc[:, TC * k:TC * (k + 1)],
                        in_=xT_d[128 * k:128 * (k + 1), c0:c0 + TC],
                    )

                # ---- q/k projections (feature-major) ----
                qc = sb.tile([128, OT * TC], attn_dt, name=f"qc{c}", tag="qc")
                kc = sb.tile([128, OT * TC], attn_dt, name=f"kc{c}", tag="kc")
                for (wsb, dst) in ((wq_sb, qc), (wk_sb, kc)):
                    for o in range(OT):
                        pj = ps.tile([128, TC], f32, name=f"pj{c}{o}", tag="proj")
                        for k in range(KT):
                            nc.tensor.matmul(
                                pj,
                                lhsT=wsb[k][:, 128 * o:128 * (o + 1)],
                                rhs=xc[:, TC * k:TC * (k + 1)],
                                start=(k == 0), stop=(k == KT - 1),
                            )
                        nc.scalar.activation(dst[:, TC * o:TC * (o + 1)], pj, Ident)

                # ---- v projection (token-major, per-head 65-col blocks) ----
                vaug = []
                for im in range(IMGS_PER_CHUNK):
                    for tt in range(2):
                        tl = 128 if tt == 0 else N - 128
                        va = sb.tile([128, H * 65], attn_dt,
                                     name=f"va{c}{im}{tt}", tag="vaug", bufs=4)
                        vav = va.rearrange("p (h c) -> p h c", c=65)
                        for oc in range(2):
                            vp = ps.tile([128, 384], f32,
                                         name=f"vp{c}{im}{tt}{oc}", tag="proj")
                            for k in range(KT):
                                nc.tensor.matmul(
                                    vp[:tl],
                                    lhsT=xc[:, TC * k + N * im + 128 * tt:
                                            TC * k + N * im + 128 * tt + tl],
                                    rhs=wv_sb[k][:, 384 * oc:384 * (oc + 1)],
                                    start=(k == 0), stop=(k == KT - 1),
                                )
                            nc.vector.tensor_copy(
                                vav[:tl, 6 * oc:6 * (oc + 1), 0:64],
                                vp[:tl].rearrange("p (h c) -> p h c", c=64),
                            )
                        for h in range(H):
                            nc.gpsimd.memset(vav[:tl, h, 64:65], float(s_out[h]))
                        vaug.append(va)

                # ---- attention per image ----
                oqf_all = []
                for im in range(IMGS_PER_CHUNK):
                    pv = []
                    for it in range(2):
                        pvt = ps.tile([128, 1024], f32,
                                      name=f"pv{c}{im}{it}", tag="pv")
                        pv.append(pvt)
                    for h in range(H):
                        o, row = h // 2, (h % 2) * 64
                        base = TC * o + N * im
                        efs = []
                        for jt in range(2):
                            jl = 128 if jt == 0 else N - 128
                            sp = ps.tile([128, N], f32,
                                         name=f"sp{c}{im}{h}{jt}", tag="st")
                            nc.tensor.matmul(
                                sp[:jl],
                                lhsT=kc[row:row + 64,
                                        base + 128 * jt:base + 128 * jt + jl],
                                rhs=qc[row:row + 64, base:base + N],
                                start=True, stop=True,
                            )
                            q16 = sb.tile([128, N], i16,
                                          name=f"q16{c}{im}{h}{jt}", tag="q16", bufs=4)
                            nc.vector.tensor_scalar(
                                out=q16[:jl], in0=sp[:jl],
                                scalar1=float(hi_s_attn[h]),
                                scalar2=float(lo_s_attn[h]),
                                op0=A.min, op1=A.max,
                            )
                            ef = sb.tile([128, N], attn_dt,
                                         name=f"ef{c}{im}{h}{jt}", tag="ef", bufs=4)
                            nc.scalar.activation(ef[:jl], q16[:jl], Exp,
                                                 scale=float(s_attn[h]))
                            efs.append(ef)
                        for it in range(2):
                            il = 128 if it == 0 else N - 128
                            off = _head_off(h)
                            for jt in range(2):
                                jl = 128 if jt == 0 else N - 128
                                nc.tensor.matmul(
                                    pv[it][:il, off:off + 65],
                                    lhsT=efs[jt][:jl, 128 * it:128 * it + il],
                                    rhs=vaug[2 * im + jt].rearrange(
                                        "p (h c) -> p h c", c=65)[:jl, h, :],
                                    start=(jt == 0), stop=(jt == 1),
                                )
                    # normalization + out-quant
                    for it in range(2):
                        il = 128 if it == 0 else N - 128
                        inv = sb.tile([128, H], f32, name=f"inv{c}{im}{it}",
                                      tag="inv", bufs=4)
                        bank0 = pv[it][:, 0:65 * 7].rearrange(
                            "p (h c) -> p h c", c=65)
                        bank1 = pv[it][:, 512:512 + 65 * 5].rearrange(
                            "p (h c) -> p h c", c=65)
                        nc.vector.reciprocal(inv[:il, 0:7], bank0[:il, :, 64])
                        nc.vector.reciprocal(inv[:il, 7:12], bank1[:il, :, 64])
                        tmp = sb.tile([128, D], f32, name=f"oqt{c}{im}{it}",
                                      tag="oqt")
                        oqi = sb.tile([128, D], i16, name=f"oqi{c}{im}{it}",
                                      tag="oqi")
                        for h in range(H):
                            off = _head_off(h)
                            nc.vector.tensor_scalar(
                                out=tmp[:il, 64 * h:64 * (h + 1)],
                                in0=pv[it][:il, off:off + 64],
                                scalar1=inv[:il, h:h + 1],
                                scalar2=float(hi_s_out[h]),
                                op0=A.mult, op1=A.min,
                            )
                            nc.vector.tensor_scalar(
                                out=oqi[:il, 64 * h:64 * (h + 1)],
                                in0=tmp[:il, 64 * h:64 * (h + 1)],
                                scalar1=float(lo_s_out[h]), scalar2=None,
                                op0=A.max,
                            )
                        oqf = sb.tile([128, D], attn_dt, name=f"oqf{c}{im}{it}",
                                      tag="oqf", bufs=4)
                        nc.vector.tensor_copy(oqf[:il], oqi[:il])
                        oqf_all.append(oqf)

                # ---- transpose Oq -> feature-major OT chunk ----
                otc = sb.tile([128, KT * TC], attn_dt, name=f"otc{c}", tag="otc",
                              bufs=1)
                for k in range(KT):
                    tp = ps.tile([128, TC], f32, name=f"tp{c}{k}", tag="proj")
                    for im in range(IMGS_PER_CHUNK):
                        for it in range(2):
                            il = 128 if it == 0 else N - 128
                            coff = N * im + 128 * it
                            nc.tensor.transpose(
                                tp[:, coff:coff + il],
                                oqf_all[2 * im + it][:il, 128 * k:128 * (k + 1)],
                                ident[:il, :il],
                            )
                    nc.scalar.activation(otc[:, TC * k:TC * (k + 1)], tp, Ident)

                # ---- output projection ----
                for o in range(OT):
                    op_ = ps.tile([128, TC], f32, name=f"op{c}{o}", tag="proj")
                    for k in range(KT):
                        nc.tensor.matmul(
                            op_,
                            lhsT=wo_sb[k][:, 128 * o:128 * (o + 1)],
                            rhs=otc[:, TC * k:TC * (k + 1)],
                            start=(k == 0), stop=(k == KT - 1),
                        )
                    osb = sb.tile([128, TC], f32, name=f"osb{c}{o}", tag="osb",
                                  bufs=3)
                    nc.scalar.activation(osb, op_, Ident, bias=bo_sb[:, o:o + 1])
                    nc.sync.dma_start(
                        out=out_d[128 * o:128 * (o + 1), c0:c0 + TC], in_=osb
                    )
    nc.compile()
    return nc


def _prepare_host_inputs(x, Wq, Wk, Wv, Wo, bo,
                         qmin_attn, qmax_attn, qmin_out, qmax_out, variant):
    """Returns (in_maps list per core, qparam tuple)."""
    f = np.float32
    alpha = np.float32(D ** -0.5)
    s_attn = ((qmax_attn - qmin_attn) / Q_LEVELS).astype(f)
    s_out = ((qmax_out - qmin_out) / Q_LEVELS).astype(f)
    hi_s_attn = (qmax_attn / s_attn).astype(f)
    lo_s_attn = (qmin_attn / s_attn).astype(f)
    hi_s_out = (qmax_out / s_out).astype(f)
    lo_s_out = (qmin_out / s_out).astype(f)

    head_of_o = np.arange(D) // DH
    wqts = np.ascontiguousarray(
        (Wq * (alpha / s_attn[head_of_o])[:, None]).T).astype(f)
    wkt = np.ascontiguousarray(Wk.T).astype(f)
    wvt = np.ascontiguousarray(Wv.T).astype(f)
    wots = np.ascontiguousarray((Wo * s_out[head_of_o][None, :]).T).astype(f)
    bof = (bo + Wo @ qmin_out[head_of_o]).astype(f)

    if variant == "bf16":
        import ml_dtypes
        adt = ml_dtypes.bfloat16
        wots_c, ident = wots.astype(adt), np.eye(128, dtype=adt)
    else:
        wots_c, ident = wots, np.eye(128, dtype=f)

    in_maps = []
    for i in range(NCORES):
        xs = np.ascontiguousarray(
            x[BPC * i:BPC * (i + 1)].reshape(T, D).T).astype(f)
        in_maps.append(dict(xT=xs, wqts=wqts, wkt=wkt, wvt=wvt, wots=wots_c,
                            bof=bof, ident=ident))
    qparams = (hi_s_attn, lo_s_attn, s_attn, hi_s_out, lo_s_out, s_out)
    return in_maps, qparams


class _Runner:
    """Compiled SPMD executable over 8 cores (PJRT path, jit cached)."""

    def __init__(self, nc):
        import jax
        import concourse.mybir as mybir
        from concourse import bass2jax
        from jax.sharding import Mesh, PartitionSpec
        from jax.experimental.shard_map import shard_map

        bass2jax.install_neuronx_cc_hook()
        self.nc = nc
        assert nc.dbg_addr is None
        partition_name = (nc.partition_id_tensor.name
                          if nc.partition_id_tensor else None)

        in_names, out_names, out_avals, zero_outs = [], [], [], []
        for alloc in nc.m.functions[0].allocations:
            if not isinstance(alloc, mybir.MemoryLocationSet):
                continue
            name = alloc.memorylocations[0].name
            if alloc.kind == "ExternalInput":
                if name != partition_name:
                    in_names.append(name)
            elif alloc.kind == "ExternalOutput":
                shape = tuple(alloc.tensor_shape)
                dtype = mybir.dt.np(alloc.dtype)
                out_names.append(name)
                out_avals.append(jax.core.ShapedArray(shape, dtype))
                zero_outs.append(np.zeros(shape, dtype))
        self.in_names, self.out_names = in_names, out_names
        self.out_avals, self.zero_outs = out_avals, zero_outs
        n_params, n_outs = len(in_names), len(out_avals)
        all_names = list(in_names) + list(out_names)
        if partition_name is not None:
            all_names.append(partition_name)
        all_names = tuple(all_names)

        def _body(*args):
            operands = list(args)
            if partition_name is not None:
                operands.append(bass2jax.partition_id_tensor())
            outs = bass2jax._bass_exec_p.bind(
                *operands,
                out_avals=tuple(out_avals),
                in_names=all_names,
                out_names=tuple(out_names),
                lowering_input_output_aliases=(),
                sim_require_finite=True,
                sim_require_nnan=True,
                nc=nc,
            )
            return tuple(outs)

        devices = jax.devices()[:NCORES]
        mesh = Mesh(np.asarray(devices), ("core",))
        self.mesh = mesh
        self.spec = PartitionSpec("core")
        self.sharded = jax.jit(
            shard_map(_body, mesh=mesh,
                      in_specs=(PartitionSpec("core"),) * (n_params + n_outs),
                      out_specs=(PartitionSpec("core"),) * n_outs,
                      check_rep=False),
            donate_argnums=tuple(range(n_params, n_params + n_outs)),
            keep_unused=True,
        )
        import jax.numpy as jnp
        from jax.sharding import NamedSharding
        zshardings = tuple(NamedSharding(mesh, self.spec) for _ in zero_outs)
        zshapes = [(NCORES * z.shape[0], *z.shape[1:]) for z in zero_outs]
        zdtypes = [z.dtype for z in zero_outs]
        self.zeros_fn = jax.jit(
            lambda: tuple(jnp.zeros(s, d) for s, d in zip(zshapes, zdtypes)),
            out_shardings=zshardings,
        )

    def device_put_inputs(self, concat_in):
        import jax
        from jax.sharding import NamedSharding
        sh = NamedSharding(self.mesh, self.spec)
        return [jax.device_put(a, sh) for a in concat_in]

    def concat_inputs(self, in_maps):
        return [np.concatenate([np.asarray(m[name]) for m in in_maps], axis=0)
                for name in self.in_names]

    def run_raw(self, concat_in):
        return self.sharded(*concat_in, *self.zeros_fn())

    def run(self, in_maps):
        out_arrs = self.run_raw(self.concat_inputs(in_maps))
        return [
            {name: np.asarray(out_arrs[i]).reshape(
                NCORES, *self.out_avals[i].shape)[c]
             for i, name in enumerate(self.out_names)}
            for c in range(NCORES)
        ]


def get_runner(qparams, variant):
    key = (variant,) + tuple(p.tobytes() for p in qparams)
    if key not in _RUNNER_CACHE:
        _RUNNER_CACHE[key] = _Runner(_build_program(*qparams, variant))
    return _RUNNER_CACHE[key]


def kernel(x, Wq, Wk, Wv, Wo, bo, qmin_attn, qmax_attn, qmin_out, qmax_out):
    variant = os.environ.get("KVAR", "f32")
    in_maps, qparams = _prepare_host_inputs(
        np.asarray(x, np.float32), np.asarray(Wq, np.float32),
        np.asarray(Wk, np.float32), np.asarray(Wv, np.float32),
        np.asarray(Wo, np.float32), np.asarray(bo, np.float32),
        np.asarray(qmin_attn, np.float32), np.asarray(qmax_attn, np.float32),
        np.asarray(qmin_out, np.float32), np.asarray(qmax_out, np.float32),
        variant,
    )
    runner = get_runner(qparams, variant)
    results = runner.run(in_maps)
    out = np.empty((B, N, D), np.float32)
    for i in range(NCORES):
        out[BPC * i:BPC * (i + 1)] = results[i]["outT"].T.reshape(BPC, N, D)
    kernel.last_runner = runner
    kernel.last_in_maps = in_maps
    return out


# revision 16
# speedup vs baseline: 5862.6059x; 63.8305x over previous
"""Trainium2 Bass kernel for quantized multi-head attention (ViT-shape).

Computation (per reference):
  q/k/v = x @ W{q,k,v}.T ; per-head scores = (q k^T) * D^-0.5 ;
  fake_quant_per_head(scores) ; softmax ; out = attn @ v ;
  fake_quant_per_head(out) ; merge heads ; out @ Wo.T + bo.

Sharding: data-parallel over batch, 8 images per core on 8 NeuronCores.

Key device-side design (per core, 8 images = 1576 tokens):
  - All weights host-transposed to [d_in, d_out] layout; quant scale factors
    folded into Wq (alpha/s_attn per head) and Wo (s_out per head); quant
    zero-offset (lo) folded into the output bias host-side.
  - q,k computed feature-major qT/kT [768, t] (heads on partitions) so the
    scores matmul contracts over d_head directly.
  - Scores computed transposed: ST[j, i] (j = key token on partitions). The
    fake-quant is ONE 2-op tensor_scalar: clip(min,max) + int16-convert
    (convert truncates toward zero == torch trunc). exp via ACT from int16
    with scale=s (the +lo offset cancels in softmax).
  - softmax denominator comes free from the P@V matmul: v is stored with an
    extra per-head column holding s_out[h]; column 64 of the PV output is
    s_out*sum_j(E) which is exactly the reciprocal argument needed for the
    normalized+pre-divided out-quant.
  - out-quant: TS(mult 1/denom', min hi/s) -> TS(max lo/s -> int16) ->
    convert; integer-valued Oq feeds the output projection; PE transpose
    (with identity) converts Oq to feature-major for the Wo matmul.
  - fp32 everywhere by default; KVAR env selects faster dtype variants.
"""

import os
import numpy as np

B, N, D, H = 64, 197, 768, 12
DH = D // H  # 64
NCORES = 8
BPC = B // NCORES          # 8 images per core
T = BPC * N                # 1576 tokens per core
IMGS_PER_CHUNK = 2
NCHUNK = BPC // IMGS_PER_CHUNK  # 4
TC = IMGS_PER_CHUNK * N    # 394 tokens per chunk
KT = D // 128              # 6 d-tiles
OT = D // 128              # 6 o-tiles
Q_LEVELS = 255

_RUNNER_CACHE = {}


def _head_off(h):
    # per-image wide PV psum [128, 1024] (2 banks): heads 0-6 in bank 0,
    # heads 7-11 in bank 1 (a 65-wide block may not cross a 512-f32 bank).
    return 65 * h if h < 7 else 512 + 65 * (h - 7)


def _build_program(hi_s_attn, lo_s_attn, s_attn, hi_s_out, lo_s_out, s_out, variant,
                   reps=1):
    import concourse.bass as bass
    import concourse.bacc as bacc
    import concourse.mybir as mybir
    from concourse.tile import TileContext

    f32 = mybir.dt.float32
    f32r = mybir.dt.float32r
    bf16 = mybir.dt.bfloat16
    i16 = mybir.dt.int16

    # w_dt: dtype of DMA-loaded projection operands (wq/wk/wv, xT).
    # attn_dt: dtype of on-device-written matmul operands (q/k/E/v/Oq/OT)
    # and of wo (wo must match OT for the output projection).
    if variant == "f32":
        w_dt, attn_dt = f32, f32
    elif variant == "f32r":
        w_dt, attn_dt = f32r, f32
    elif variant == "bf16":
        w_dt, attn_dt = f32r, bf16
    else:
        raise ValueError(variant)
    wo_dt = attn_dt

    nc = bacc.Bacc("TRN2", target_bir_lowering=False, debug=False)

    xT_d = nc.dram_tensor("xT", [D, T], w_dt, kind="ExternalInput").ap()
    wq_d = nc.dram_tensor("wqts", [D, D], w_dt, kind="ExternalInput").ap()
    wk_d = nc.dram_tensor("wkt", [D, D], w_dt, kind="ExternalInput").ap()
    wv_d = nc.dram_tensor("wvt", [D, D], w_dt, kind="ExternalInput").ap()
    wo_d = nc.dram_tensor("wots", [D, D], wo_dt, kind="ExternalInput").ap()
    bo_d = nc.dram_tensor("bof", [D], f32, kind="ExternalInput").ap()
    id_d = nc.dram_tensor("ident", [128, 128], attn_dt, kind="ExternalInput").ap()
    out_d = nc.dram_tensor("outT", [D, T], f32, kind="ExternalOutput").ap()

    Exp = mybir.ActivationFunctionType.Exp
    Ident = mybir.ActivationFunctionType.Identity
    A = mybir.AluOpType

    with TileContext(nc) as tc:
        with (
            tc.tile_pool(name="const", bufs=1) as cpool,
            tc.tile_pool(name="sb", bufs=2) as sb,
            tc.tile_pool(name="ps", bufs=2, space="PSUM") as ps,
        ):
            # ---- resident constants ----
            wq_sb, wk_sb, wv_sb, wo_sb = [], [], [], []
            for k in range(KT):
                t_q = cpool.tile([128, D], w_dt, name=f"wq{k}")
                nc.sync.dma_start(out=t_q, in_=wq_d[128 * k:128 * (k + 1), :])
                wq_sb.append(t_q)
                t_k = cpool.tile([128, D], w_dt, name=f"wk{k}")
                nc.sync.dma_start(out=t_k, in_=wk_d[128 * k:128 * (k + 1), :])
                wk_sb.append(t_k)
                t_v = cpool.tile([128, D], w_dt, name=f"wv{k}")
                nc.sync.dma_start(out=t_v, in_=wv_d[128 * k:128 * (k + 1), :])
                wv_sb.append(t_v)
                t_o = cpool.tile([128, D], wo_dt, name=f"wo{k}")
                nc.sync.dma_start(out=t_o, in_=wo_d[128 * k:128 * (k + 1), :])
                wo_sb.append(t_o)
            bo_sb = cpool.tile([128, OT], f32, name="bo")
            for k in range(OT):
                nc.sync.dma_start(
                    out=bo_sb[:, k:k + 1],
                    in_=bo_d[128 * k:128 * (k + 1)].rearrange("(p o) -> p o", o=1),
                )
            ident = cpool.tile([128, 128], attn_dt, name="ident")
            nc.sync.dma_start(out=ident, in_=id_d)

            import contextlib
            rep_ctx = tc.For_i(0, reps, 1) if reps > 1 else contextlib.nullcontext()
            with rep_ctx:
                _emit_body(nc, tc, sb, ps, locals())
    nc.compile()
    return nc


def _emit_body(nc, tc, sb, ps, env):
    import concourse.mybir as mybir
    xc_ = None  # placeholder to appease linters
    # unpack closure values
    (xT_d, out_d, wq_sb, wk_sb, wv_sb, wo_sb, bo_sb, ident) = (
        env["xT_d"], env["out_d"], env["wq_sb"], env["wk_sb"], env["wv_sb"],
        env["wo_sb"], env["bo_sb"], env["ident"])
    (w_dt, attn_dt, f32, i16) = env["w_dt"], env["attn_dt"], env["f32"], env["i16"]
    (hi_s_attn, lo_s_attn, s_attn) = env["hi_s_attn"], env["lo_s_attn"], env["s_attn"]
    (hi_s_out, lo_s_out, s_out) = env["hi_s_out"], env["lo_s_out"], env["s_out"]
    Exp, Ident, A = env["Exp"], env["Ident"], env["A"]
    if True:
        if True:
            for c in range(NCHUNK):
                c0 = TC * c
                # ---- load xT chunk ----
                xc = sb.tile([128, KT * TC], w_dt, name=f"xc{c}", tag="xc")
                for k in range(KT):
                    nc.sync.dma_start(
                        out=xc[:, TC * k:TC * (k + 1)],
                        in_=xT_d[128 * k:128 * (k + 1), c0:c0 + TC],
                    )

                # ---- q/k projections (feature-major) ----
                qc = sb.tile([128, OT * TC], attn_dt, name=f"qc{c}", tag="qc")
                kc = sb.tile([128, OT * TC], attn_dt, name=f"kc{c}", tag="kc")
                for (wsb, dst) in ((wq_sb, qc), (wk_sb, kc)):
                    for o in range(OT):
                        pj = ps.tile([128, TC], f32, name=f"pj{c}{o}", tag="proj")
                        for k in range(KT):
                            nc.tensor.matmul(
                                pj,
                                lhsT=wsb[k][:, 128 * o:128 * (o + 1)],
                                rhs=xc[:, TC * k:TC * (k + 1)],
                                start=(k == 0), stop=(k == KT - 1),
                            )
                        nc.scalar.activation(dst[:, TC * o:TC * (o + 1)], pj, Ident)

                # ---- v projection (token-major, per-head 65-col blocks) ----
                vaug = []
                for im in range(IMGS_PER_CHUNK):
                    for tt in range(2):
                        tl = 128 if tt == 0 else N - 128
                        va = sb.tile([128, H * 65], attn_dt,
                                     name=f"va{c}{im}{tt}", tag="vaug", bufs=4)
                        vav = va.rearrange("p (h c) -> p h c", c=65)
                        for oc in range(2):
                            vp = ps.tile([128, 384], f32,
                                         name=f"vp{c}{im}{tt}{oc}", tag="proj")
                            for k in range(KT):
                                nc.tensor.matmul(
                                    vp[:tl],
                                    lhsT=xc[:, TC * k + N * im + 128 * tt:
                                            TC * k + N * im + 128 * tt + tl],
                                    rhs=wv_sb[k][:, 384 * oc:384 * (oc + 1)],
                                    start=(k == 0), stop=(k == KT - 1),
                                )
                            nc.vector.tensor_copy(
                                vav[:tl, 6 * oc:6 * (oc + 1), 0:64],
                                vp[:tl].rearrange("p (h c) -> p h c", c=64),
                            )
                        for h in range(H):
                            nc.gpsimd.memset(vav[:tl, h, 64:65], float(s_out[h]))
                        vaug.append(va)

                # ---- attention per image ----
                oqf_all = []
                for im in range(IMGS_PER_CHUNK):
                    pv = []
                    for it in range(2):
                        pvt = ps.tile([128, 1024], f32,
                                      name=f"pv{c}{im}{it}", tag="pv")
                        pv.append(pvt)
                    for h in range(H):
                        o, row = h // 2, (h % 2) * 64
                        base = TC * o + N * im
                        efs = []
                        for jt in range(2):
                            jl = 128 if jt == 0 else N - 128
                            sp = ps.tile([128, N], f32,
                                         name=f"sp{c}{im}{h}{jt}", tag="st")
                            nc.tensor.matmul(
                                sp[:jl],
                                lhsT=kc[row:row + 64,
                                        base + 128 * jt:base + 128 * jt + jl],
                                rhs=qc[row:row + 64, base:base + N],
                                start=True, stop=True,
                            )
                            q16 = sb.tile([128, N], i16,
                                          name=f"q16{c}{im}{h}{jt}", tag="q16", bufs=4)
                            nc.vector.tensor_scalar(
                                out=q16[:jl], in0=sp[:jl],
                                scalar1=float(hi_s_attn[h]),
                                scalar2=float(lo_s_attn[h]),
                                op0=A.min, op1=A.max,
                            )
                            ef = sb.tile([128, N], attn_dt,
                                         name=f"ef{c}{im}{h}{jt}", tag="ef", bufs=4)
                            nc.scalar.activation(ef[:jl], q16[:jl], Exp,
                                                 scale=float(s_attn[h]))
                            efs.append(ef)
                        for it in range(2):
                            il = 128 if it == 0 else N - 128
                            off = _head_off(h)
                            for jt in range(2):
                                jl = 128 if jt == 0 else N - 128
                                nc.tensor.matmul(
                                    pv[it][:il, off:off + 65],
                                    lhsT=efs[jt][:jl, 128 * it:128 * it + il],
                                    rhs=vaug[2 * im + jt].rearrange(
                                        "p (h c) -> p h c", c=65)[:jl, h, :],
                                    start=(jt == 0), stop=(jt == 1),
                                )
                    # normalization + out-quant
                    for it in range(2):
                        il = 128 if it == 0 else N - 128
                        inv = sb.tile([128, H], f32, name=f"inv{c}{im}{it}",
                                      tag="inv", bufs=4)
                        bank0 = pv[it][:, 0:65 * 7].rearrange(
                            "p (h c) -> p h c", c=65)
                        bank1 = pv[it][:, 512:512 + 65 * 5].rearrange(
                            "p (h c) -> p h c", c=65)
                        nc.vector.reciprocal(inv[:il, 0:7], bank0[:il, :, 64])
                        nc.vector.reciprocal(inv[:il, 7:12], bank1[:il, :, 64])
                        tmp = sb.tile([128, D], f32, name=f"oqt{c}{im}{it}",
                                      tag="oqt")
                        oqi = sb.tile([128, D], i16, name=f"oqi{c}{im}{it}",
                                      tag="oqi")
                        for h in range(H):
                            off = _head_off(h)
                            nc.vector.tensor_scalar(
                                out=tmp[:il, 64 * h:64 * (h + 1)],
                                in0=pv[it][:il, off:off + 64],
                                scalar1=inv[:il, h:h + 1],
                                scalar2=float(hi_s_out[h]),
                                op0=A.mult, op1=A.min,
                            )
                            nc.vector.tensor_scalar(
                                out=oqi[:il, 64 * h:64 * (h + 1)],
                                in0=tmp[:il, 64 * h:64 * (h + 1)],
                                scalar1=float(lo_s_out[h]), scalar2=None,
                                op0=A.max,
                            )
                        oqf = sb.tile([128, D], attn_dt, name=f"oqf{c}{im}{it}",
                                      tag="oqf", bufs=4)
                        nc.vector.tensor_copy(oqf[:il], oqi[:il])
                        oqf_all.append(oqf)

                # ---- transpose Oq -> feature-major OT chunk ----
                otc = sb.tile([128, KT * TC], attn_dt, name=f"otc{c}", tag="otc",
                              bufs=1)
                for k in range(KT):
                    tp = ps.tile([128, TC], f32, name=f"tp{c}{k}", tag="proj")
                    for im in range(IMGS_PER_CHUNK):
                        for it in range(2):
                            il = 128 if it == 0 else N - 128
                            coff = N * im + 128 * it
                            nc.tensor.transpose(
                                tp[:, coff:coff + il],
                                oqf_all[2 * im + it][:il, 128 * k:128 * (k + 1)],
                                ident[:il, :il],
                            )
                    nc.scalar.activation(otc[:, TC * k:TC * (k + 1)], tp, Ident)

                # ---- output projection ----
                for o in range(OT):
                    op_ = ps.tile([128, TC], f32, name=f"op{c}{o}", tag="proj")
                    for k in range(KT):
                        nc.tensor.matmul(
                            op_,
                            lhsT=wo_sb[k][:, 128 * o:128 * (o + 1)],
                            rhs=otc[:, TC * k:TC * (k + 1)],
                            start=(k == 0), stop=(k == KT - 1),
                        )
                    osb = sb.tile([128, TC], f32, name=f"osb{c}{o}", tag="osb",
                                  bufs=3)
                    nc.scalar.activation(osb, op_, Ident, bias=bo_sb[:, o:o + 1])
                    nc.sync.dma_start(
                        out=out_d[128 * o:128 * (o + 1), c0:c0 + TC], in_=osb
                    )


def _prepare_host_inputs(x, Wq, Wk, Wv, Wo, bo,
                         qmin_attn, qmax_attn, qmin_out, qmax_out, variant):
    """Returns (in_maps list per core, qparam tuple)."""
    f = np.float32
    alpha = np.float32(D ** -0.5)
    s_attn = ((qmax_attn - qmin_attn) / Q_LEVELS).astype(f)
    s_out = ((qmax_out - qmin_out) / Q_LEVELS).astype(f)
    hi_s_attn = (qmax_attn / s_attn).astype(f)
    lo_s_attn = (qmin_attn / s_attn).astype(f)
    hi_s_out = (qmax_out / s_out).astype(f)
    lo_s_out = (qmin_out / s_out).astype(f)

    head_of_o = np.arange(D) // DH
    wqts = np.ascontiguousarray(
        (Wq * (alpha / s_attn[head_of_o])[:, None]).T).astype(f)
    wkt = np.ascontiguousarray(Wk.T).astype(f)
    wvt = np.ascontiguousarray(Wv.T).astype(f)
    wots = np.ascontiguousarray((Wo * s_out[head_of_o][None, :]).T).astype(f)
    bof = (bo + Wo @ qmin_out[head_of_o]).astype(f)

    if variant == "bf16":
        import ml_dtypes
        adt = ml_dtypes.bfloat16
        wots_c, ident = wots.astype(adt), np.eye(128, dtype=adt)
    else:
        wots_c, ident = wots, np.eye(128, dtype=f)

    in_maps = []
    for i in range(NCORES):
        xs = np.ascontiguousarray(
            x[BPC * i:BPC * (i + 1)].reshape(T, D).T).astype(f)
        in_maps.append(dict(xT=xs, wqts=wqts, wkt=wkt, wvt=wvt, wots=wots_c,
                            bof=bof, ident=ident))
    qparams = (hi_s_attn, lo_s_attn, s_attn, hi_s_out, lo_s_out, s_out)
    return in_maps, qparams


class _Runner:
    """Compiled SPMD executable over 8 cores (PJRT path, jit cached)."""

    def __init__(self, nc):
        import jax
        import concourse.mybir as mybir
        from concourse import bass2jax
        from jax.sharding import Mesh, PartitionSpec
        from jax.experimental.shard_map import shard_map

        bass2jax.install_neuronx_cc_hook()
        self.nc = nc
        assert nc.dbg_addr is None
        partition_name = (nc.partition_id_tensor.name
                          if nc.partition_id_tensor else None)

        in_names, out_names, out_avals, zero_outs = [], [], [], []
        for alloc in nc.m.functions[0].allocations:
            if not isinstance(alloc, mybir.MemoryLocationSet):
                continue
            name = alloc.memorylocations[0].name
            if alloc.kind == "ExternalInput":
                if name != partition_name:
                    in_names.append(name)
            elif alloc.kind == "ExternalOutput":
                shape = tuple(alloc.tensor_shape)
                dtype = mybir.dt.np(alloc.dtype)
                out_names.append(name)
                out_avals.append(jax.core.ShapedArray(shape, dtype))
                zero_outs.append(np.zeros(shape, dtype))
        self.in_names, self.out_names = in_names, out_names
        self.out_avals, self.zero_outs = out_avals, zero_outs
        n_params, n_outs = len(in_names), len(out_avals)
        all_names = list(in_names) + list(out_names)
        if partition_name is not None:
            all_names.append(partition_name)
        all_names = tuple(all_names)

        def _body(*args):
            operands = list(args)
            if partition_name is not None:
                operands.append(bass2jax.partition_id_tensor())
            outs = bass2jax._bass_exec_p.bind(
                *operands,
                out_avals=tuple(out_avals),
                in_names=all_names,
                out_names=tuple(out_names),
                lowering_input_output_aliases=(),
                sim_require_finite=True,
                sim_require_nnan=True,
                nc=nc,
            )
            return tuple(outs)

        devices = jax.devices()[:NCORES]
        mesh = Mesh(np.asarray(devices), ("core",))
        self.mesh = mesh
        self.spec = PartitionSpec("core")
        self.sharded = jax.jit(
            shard_map(_body, mesh=mesh,
                      in_specs=(PartitionSpec("core"),) * (n_params + n_outs),
                      out_specs=(PartitionSpec("core"),) * n_outs,
                      check_rep=False),
            donate_argnums=tuple(range(n_params, n_params + n_outs)),
            keep_unused=True,
        )
        import jax.numpy as jnp
        from jax.sharding import NamedSharding
        zshardings = tuple(NamedSharding(mesh, self.spec) for _ in zero_outs)
        zshapes = [(NCORES * z.shape[0], *z.shape[1:]) for z in zero_outs]
        zdtypes = [z.dtype for z in zero_outs]
        self.zeros_fn = jax.jit(
            lambda: tuple(jnp.zeros(s, d) for s, d in zip(zshapes, zdtypes)),
            out_shardings=zshardings,
        )

    def device_put_inputs(self, concat_in):
        import jax
        from jax.sharding import NamedSharding
        sh = NamedSharding(self.mesh, self.spec)
        return [jax.device_put(a, sh) for a in concat_in]

    def concat_inputs(self, in_maps):
        return [np.concatenate([np.asarray(m[name]) for m in in_maps], axis=0)
                for name in self.in_names]

    def run_raw(self, concat_in):
        return self.sharded(*concat_in, *self.zeros_fn())

    def run(self, in_maps):
        out_arrs = self.run_raw(self.concat_inputs(in_maps))
        return [
            {name: np.asarray(out_arrs[i]).reshape(
                NCORES, *self.out_avals[i].shape)[c]
             for i, name in enumerate(self.out_names)}
            for c in range(NCORES)
        ]


def get_runner(qparams, variant):
    key = (variant,) + tuple(p.tobytes() for p in qparams)
    if key not in _RUNNER_CACHE:
        _RUNNER_CACHE[key] = _Runner(_build_program(*qparams, variant))
    return _RUNNER_CACHE[key]


def kernel(x, Wq, Wk, Wv, Wo, bo, qmin_attn, qmax_attn, qmin_out, qmax_out):
    variant = os.environ.get("KVAR", "f32")
    in_maps, qparams = _prepare_host_inputs(
        np.asarray(x, np.float32), np.asarray(Wq, np.float32),
        np.asarray(Wk, np.float32), np.asarray(Wv, np.float32),
        np.asarray(Wo, np.float32), np.asarray(bo, np.float32),
        np.asarray(qmin_attn, np.float32), np.asarray(qmax_attn, np.float32),
        np.asarray(qmin_out, np.float32), np.asarray(qmax_out, np.float32),
        variant,
    )
    runner = get_runner(qparams, variant)
    results = runner.run(in_maps)
    out = np.empty((B, N, D), np.float32)
    for i in range(NCORES):
        out[BPC * i:BPC * (i + 1)] = results[i]["outT"].T.reshape(BPC, N, D)
    kernel.last_runner = runner
    kernel.last_in_maps = in_maps
    return out
